# revision 46
# baseline (speedup 1.0000x reference)
"""Trainium2 Bass kernel for nn_MiniMHCLM (moe_routing).

Strategy (8 NeuronCores, SPMD, no collectives):
  - vocab-sharded head matmul: core i holds w_head rows [i*VS:(i+1)*VS]
    (host-sliced, zero-padded to uniform VS) transposed to k-major bf16;
    it computes logits for all 4096 tokens x its vocab slice and the host
    concatenates along vocab.
  - token embeddings are pre-gathered AND pre-transposed on the host into
    a k-major, group-packed layout so every device load is one contiguous
    DMA descriptor per tile.
  - EVERY bulk SBUF destination is split so each tile is written by
    exactly ONE DMA descriptor: w_head lives in 14 tiles (2 kc-halves x
    7 vocab stripes), x in 2 half-tiles per token group, the coefficient
    planes in 3 tiles. Tile-granular WAW tracking otherwise serializes
    the descriptors AND blocks the issuing engine's instruction stream.
  - per-token coeffs: phi-stationary matmul gives logits^T [24, T]; a
    col-tiled ones-matmul (PE column group 1, concurrent with the phi
    stream) produces sum(x^2) at PSUM partition 32; one PE transpose per
    128-token chunk moves logits+sumsq to token-major for the RMS scale,
    sigmoid/exp and 5 Sinkhorn iterations (converged vs the reference's
    8 to ~1e-3 absolute, far below the bf16 noise floor).
  - mixing runs transposed: coeffs are PE-transposed back to [24, T],
    bounced through DRAM in three pieces (pre/post first) and broadcast
    across partitions with stride-0 DMA reads; x_merge^T is built with
    tree-structured DVE multiply-adds (gpsimd computes the independent
    f_out*post term) and feeds the head matmul as stationary operand.
  - head matmul in bf16 with fp32 PSUM; PSUM evacuated by ACT copies to
    bf16 and DMA'd to DRAM bf16 (host converts to fp32).
  - flat software pipeline over 512-token groups with a 2-group prologue
    so the PE stream runs dense shortly after start with no ramp
    starvation and no HAM re-throttling.
  - DMA queue split: sync queue carries x/consts/w_head-stripes-0..3 and
    all output writes; scalar queue carries w_head stripes 4..6 and the
    coefficient bounces.
"""

import numpy as np

HC, C, TMAX = 4, 256, 8
TMAX_K = 5
RMS_EPS, PRE_EPS, SINK_EPS, POST_MULT = 1e-6, 1e-4, 1e-6, 2.0
VOCAB = 50257
B, S = 2, 2048
K = HC * C            # 1024
M = HC * HC + 2 * HC  # 24
NKC = K // 128        # 8 k-chunks
NCORES = 8
NT = B * S            # 4096
VS = 6283             # vocab rows per core (8*6283 = 50264 >= 50257)
VW = 512
NV = (VS + VW - 1) // VW          # 13 head tiles (12x512 + 139)
SW = 2 * VW                       # w stripe width (vocab cols)
NSTR = (VS + SW - 1) // SW        # 7 stripes
SCW = [min(SW, VS - s * SW) for s in range(NSTR)]
# wv4 col offset per (stripe, kc-half): each block is [128, 4*cw]
OFFW = {}
_off = 0
for _s in range(NSTR):
    for _h in range(2):
        OFFW[(_s, _h)] = _off
        _off += 4 * SCW[_s]
SCS = [384] + [512] * 7 + [128]   # token groups, sum = NT
OFF = [sum(SCS[:i]) for i in range(len(SCS))]
NG = len(SCS)
assert sum(SCS) == NT


def _build():
    from contextlib import ExitStack
    from concourse import bass, bacc, mybir
    import concourse.tile as tile
    from concourse.masks import make_identity

    f32 = mybir.dt.float32
    bf16 = mybir.dt.bfloat16
    f8e4 = mybir.dt.float8e4
    AX = mybir.AxisListType
    OP = mybir.AluOpType
    AF = mybir.ActivationFunctionType

    nc = bacc.Bacc(target_bir_lowering=False)
    xt_p = nc.declare_dram_parameter("xt", [128, NKC * NT], bf16, False)
    wvt_p = nc.declare_dram_parameter("wvt", [128, NKC * VS], bf16, False)
    wit_p = nc.declare_dram_parameter("wit", [128, 2 * C], bf16, False)
    phi_p = nc.declare_dram_parameter("phi", [128, NKC * M], bf16, False)
    b_p = nc.declare_dram_parameter("b", [1, M], f32, False)
    al_p = nc.declare_dram_parameter("al", [1, 3], f32, False)
    out_p = nc.declare_dram_parameter("out", [NT, VS], bf16, True)

    with ExitStack() as ctx:
        tc = ctx.enter_context(tile.TileContext(nc))
        const = ctx.enter_context(tc.tile_pool(name="const", bufs=1))
        wtp = ctx.enter_context(tc.tile_pool(name="wtp", bufs=1))
        xtp = ctx.enter_context(tc.tile_pool(name="xtp", bufs=4))
        lgp = ctx.enter_context(tc.tile_pool(name="lgp", bufs=2))
        lgs = ctx.enter_context(tc.tile_pool(name="lgs", bufs=1))
        cfp = ctx.enter_context(tc.tile_pool(name="cfp", bufs=2))
        plp = ctx.enter_context(tc.tile_pool(name="plp", bufs=1))
        mxp = ctx.enter_context(tc.tile_pool(name="mxp", bufs=2))
        xmp = ctx.enter_context(tc.tile_pool(name="xmp", bufs=3))
        wkp = ctx.enter_context(tc.tile_pool(name="wkp", bufs=1))
        t4p = ctx.enter_context(tc.tile_pool(name="t4p", bufs=2))
        x2p = ctx.enter_context(tc.tile_pool(name="x2p", bufs=1))
        stp = ctx.enter_context(tc.tile_pool(name="stp", bufs=2))
        psh = ctx.enter_context(tc.tile_pool(name="psh", bufs=5, space="PSUM"))
        psa = ctx.enter_context(tc.tile_pool(name="psa", bufs=2, space="PSUM"))
        pst = ctx.enter_context(tc.tile_pool(name="pst", bufs=1, space="PSUM"))
        drp = ctx.enter_context(tc.tile_pool(name="drp", bufs=2, space="DRAM"))

        # ---------------- constants (tiny, first) ----------------------
        identf = const.tile([128, 128], f32)
        make_identity(nc, identf[:])

        cst = const.tile([128, 2], f32)
        nc.vector.memset(cst[:, 0:1], 0.0)
        nc.vector.memset(cst[:, 1:2], RMS_EPS)
        zero_b = cst[:, 0:1]
        eps_b = cst[:, 1:2]

        ones = const.tile([128, 1], bf16)
        nc.vector.memset(ones[:], 1.0)

        phi_sb = const.tile([128, NKC * M], bf16)
        nc.sync.dma_start(out=phi_sb[:], in_=phi_p[:, :])
        b_bc = const.tile([128, M], f32)
        nc.sync.dma_start(out=b_bc[:], in_=b_p[0:1, :].to_broadcast([128, M]))
        al_bc = const.tile([128, 3], f32)
        nc.sync.dma_start(out=al_bc[:], in_=al_p[0:1, :].to_broadcast([128, 3]))
        wit_sb = const.tile([128, 2 * C], bf16)
        nc.sync.dma_start(out=wit_sb[:], in_=wit_p[:, :])

        # ---------------- input prefetch (2 tiles, 1 descriptor each) --
        xt_tiles = {}

        def prefetch_xt(g, eng=None):
            gt, t0 = SCS[g], OFF[g]
            halves = []
            for hh in range(2):
                xth = xtp.tile([128, 4 * gt], bf16, tag=f"xt{hh}",
                               name=f"xt{hh}_{g}")
                (eng or nc.sync).dma_start(
                    out=xth[:],
                    in_=xt_p[:, NKC * t0 + hh * 4 * gt:
                             NKC * t0 + (hh + 1) * 4 * gt])
                halves.append(xth)
            xt_tiles[g] = halves

        prefetch_xt(0)
        prefetch_xt(1)
        prefetch_xt(2)   # hoisted x2(2) consumes this early

        # ---------------- w_head^T: 14 tiles, 1 descriptor each --------
        # tile (s, h): [128, 4, cw] covering kc = 4h..4h+3 of stripe s
        wt_t = {}
        wt_sb = {}
        for s in range(NSTR):
            cw = SCW[s]
            for h in range(2):
                t = wtp.tile([128, 4 * cw], bf16, tag=f"wt_{s}_{h}")
                wt_sb[(s, h)] = t
                wt_t[(s, h)] = t[:].rearrange("p (k c) -> p k c", k=4)

        def load_wt(s, eng):
            for h in range(2):
                eng.dma_start(
                    out=wt_sb[(s, h)][:],
                    in_=wvt_p[:, OFFW[(s, h)]:OFFW[(s, h)] + 4 * cw_of(s)])

        def cw_of(s):
            return SCW[s]

        # 3-way queue split sized to arrival deadlines: s0-s2 on sync
        # (behind x0..x2), s5-s6 on the gpsimd swdge queue (issued now,
        # idle third channel), s3-s4 on scalar AFTER the g0/g1 coefficient
        # bounces (emitted at the end of the prologue).
        for s in (0, 1, 2):
            load_wt(s, nc.sync)
        for s in (5, 6):
            load_wt(s, nc.gpsimd)
        prefetch_xt(3)

        st = {}  # per-group live tiles

        def xs(g, kc, gt):
            """xtg slice for k-chunk kc: [128, gt] view."""
            return xt_tiles_sb[g][kc // 4][:, (kc % 4) * gt:(kc % 4 + 1) * gt]

        xt_tiles_sb = xt_tiles  # alias (halves list per group)

        # ---------------- pipeline stages ----------------
        x2_tiles = {}

        def stage_x2(g):
            """squares for the sumsq matmuls (DVE), hoistable so the ss
            MMs never head-of-line block the PE FIFO on a late square."""
            gt = SCS[g]
            xh = xt_tiles[g]
            x2s = []
            for q4 in range(4):
                # fp8e4 with a x256 pre-scale: keeps the squares in fp8's
                # normal range; mean over K=1024 needs only ~0.1% accuracy
                x2 = x2p.tile([128, 2 * gt], f8e4, tag=f"x2q{q4}",
                              name=f"x2_{g}_{q4}")
                src = xh[q4 // 2][:, (q4 % 2) * 2 * gt:(q4 % 2 + 1) * 2 * gt]
                nc.vector.scalar_tensor_tensor(
                    out=x2[:], in0=src, scalar=256.0, in1=src,
                    op0=OP.mult, op1=OP.mult)
                x2s.append(x2)
            x2_tiles[g] = x2s

        def stage_lg(g):
            gt, t0 = SCS[g], OFF[g]
            nch = gt // 128
            if g >= 2 and g + 2 < NG:
                # xtp has 4 buffers, so this rotates into a long-dead slot
                # and never blocks the issuing engine -- except lg(2)'s
                # (rotates into xt(0), read by mix(0) around this time):
                # that one goes via gpsimd where a wait is harmless
                prefetch_xt(g + 2, eng=nc.gpsimd if g == 2 else nc.sync)

            pslg = psa.tile([64, gt], f32, tag="pslg")
            if g not in x2_tiles:
                stage_x2(g)
            x2s = x2_tiles.pop(g)
            for kc in range(NKC):
                nc.tensor.matmul(
                    out=pslg[0:M, :],
                    lhsT=phi_sb[:, kc * M:(kc + 1) * M],
                    rhs=xs(g, kc, gt),
                    start=(kc == 0), stop=(kc == NKC - 1),
                    skip_group_check=True)
                nc.tensor.matmul(
                    out=pslg[32:33, :],
                    lhsT=ones[:],
                    rhs=x2s[kc // 2][:, (kc % 2) * gt:(kc % 2 + 1) * gt],
                    start=(kc == 0), stop=(kc == NKC - 1),
                    skip_group_check=True)

            lgsb = lgs.tile([33, gt], f32, tag="lgsb", name=f"lgsb{g}")
            nc.vector.memset(lgsb[0:33, :], 0.0)
            nc.scalar.copy(lgsb[0:M, :], pslg[0:M, :])
            nc.scalar.copy(lgsb[32:33, :], pslg[32:33, :])

            # token-major [128, nch, 24] + per-token sumsq column, via a
            # single 33-row PE transpose per 128-token chunk
            lgtm = lgp.tile([128, nch * 32], f32, tag="lgtm", name=f"lgtm{g}")
            msq = lgp.tile([128, nch], f32, tag="msq", name=f"msq{g}")
            for tcx in range(nch):
                pT = pst.tile([128, 128], f32, tag="psT")
                nc.tensor.transpose(
                    out=pT[:, 0:33],
                    in_=lgsb[0:33, tcx * 128:(tcx + 1) * 128],
                    identity=identf[0:33, 0:33])
                nc.scalar.copy(lgtm[:, tcx * 32:tcx * 32 + M], pT[:, 0:M])
                nc.scalar.copy(msq[:, tcx:tcx + 1], pT[:, 32:33])
            lgv = lgtm[:].rearrange("p (c w) -> p c w", w=32)

            scl = lgp.tile([128, nch], f32, tag="scl", name=f"scl{g}")
            nc.scalar.activation(out=scl[:], in_=msq[:],
                                 func=AF.Sqrt, scale=1.0 / (256.0 * K), bias=eps_b)
            nc.vector.reciprocal(scl[:], scl[:])
            for tcx in range(nch):
                nc.vector.tensor_scalar_mul(
                    lgv[:, tcx, 0:M], lgv[:, tcx, 0:M], scl[:, tcx:tcx + 1])
            nc.vector.tensor_tensor(
                out=lgv[:, :, 0:M], in0=lgv[:, :, 0:M],
                in1=b_bc[:][:, None, :].to_broadcast([128, nch, M]), op=OP.add)

            coefs = cfp.tile([128, nch * M], f32, tag="coefs",
                             name=f"coefs{g}")
            cfv = coefs[:].rearrange("p (c m) -> p c m", m=M)
            nc.scalar.activation(out=cfv[:, :, 16:20], in_=lgv[:, :, 0:4],
                                 func=AF.Sigmoid, bias=zero_b,
                                 scale=al_bc[:, 0:1])
            nc.vector.tensor_scalar_add(cfv[:, :, 16:20], cfv[:, :, 16:20],
                                        PRE_EPS)
            nc.scalar.activation(out=cfv[:, :, 20:24], in_=lgv[:, :, 4:8],
                                 func=AF.Sigmoid, bias=zero_b,
                                 scale=al_bc[:, 1:2])
            nc.scalar.activation(out=cfv[:, :, 0:16], in_=lgv[:, :, 8:24],
                                 func=AF.Exp, bias=zero_b, scale=al_bc[:, 2:3])

            mv4 = cfv[:, :, 0:16].rearrange("p c (o i) -> p c o i", i=4)
            mv4t = cfv[:, :, 0:16].rearrange("p c (o i) -> p c i o", i=4)
            for _ in range(TMAX_K):
                rs = wkp.tile([128, 16], f32, tag="rs")
                rsv = rs[:, 0:nch * 4].rearrange("p (c o) -> p c o", c=nch)
                nc.vector.tensor_reduce(rsv, mv4, axis=AX.X, op=OP.add)
                nc.vector.reciprocal(rs[:, 0:nch * 4], rs[:, 0:nch * 4])
                nc.vector.tensor_tensor(
                    out=mv4, in0=mv4,
                    in1=rsv[:, :, :, None].to_broadcast([128, nch, 4, 4]),
                    op=OP.mult)
                cs = wkp.tile([128, 16], f32, tag="cs")
                csv = cs[:, 0:nch * 4].rearrange("p (c i) -> p c i", c=nch)
                nc.vector.tensor_reduce(csv, mv4t, axis=AX.X, op=OP.add)
                nc.vector.reciprocal(cs[:, 0:nch * 4], cs[:, 0:nch * 4])
                nc.vector.tensor_tensor(
                    out=mv4, in0=mv4,
                    in1=csv[:, :, None, :].to_broadcast([128, nch, 4, 4]),
                    op=OP.mult)
            st[g] = dict(coefs=coefs)

        def stage_planes(g):
            """coefs -> [24, T] via PE transpose; bounce through DRAM in
            three pieces (pre/post first) and broadcast-read into three
            single-descriptor plane tiles; build x_in^T from pre planes."""
            gt = SCS[g]
            nch = gt // 128
            coefs = st[g]["coefs"]
            ctstg = cfp.tile([32, gt], bf16, tag="ctstg", name=f"ctstg{g}")
            for tcx in range(nch):
                pT = pst.tile([128, 128], f32, tag="psT")
                nc.tensor.transpose(
                    out=pT[0:M, 0:128],
                    in_=coefs[:, tcx * M:(tcx + 1) * M],
                    identity=identf[:, 0:128])
                nc.scalar.copy(
                    ctstg[0:M, tcx * 128:(tcx + 1) * 128], pT[0:M, 0:128])
            planA = plp.tile([128, 8 * gt], bf16, tag="planA",
                             name=f"planA{g}")   # planes 16..23 (pre+post)
            planB1 = plp.tile([128, 8 * gt], bf16, tag="planB1",
                              name=f"planB1{g}")  # res planes 0..7
            planB2 = plp.tile([128, 8 * gt], bf16, tag="planB2",
                              name=f"planB2{g}")  # res planes 8..15
            for nmA, rows, dst in (("A", (16, 24), planA),
                                   ("B1", (0, 8), planB1),
                                   ("B2", (8, 16), planB2)):
                dt = drp.tile([1, 8 * gt], bf16, tag=f"dt{nmA}",
                              name=f"dt{nmA}{g}")
                nc.scalar.dma_start(
                    out=dt[0:1, :].rearrange("x (c t) -> (x c) t", c=8),
                    in_=ctstg[rows[0]:rows[1], :])
                nc.scalar.dma_start(
                    out=dst[:], in_=dt[0:1, :].to_broadcast([128, 8 * gt]))

            def pl(idx):
                if idx >= 16:
                    return planA[:, (idx - 16) * gt:(idx - 15) * gt]
                if idx < 8:
                    return planB1[:, idx * gt:(idx + 1) * gt]
                return planB2[:, (idx - 8) * gt:(idx - 7) * gt]

            st[g]["pl"] = pl
            # x_in^T = sum_i h_pre[i] * x^T[i]
            xin = mxp.tile([128, 2 * gt], bf16, tag="xin", name=f"xin{g}")
            for h in range(2):
                seg = xin[:, h * gt:(h + 1) * gt]
                nc.vector.tensor_tensor(
                    out=seg, in0=xs(g, h, gt), in1=pl(16), op=OP.mult)
                t1 = wkp.tile([128, 512], bf16, tag="tm1")
                t2 = wkp.tile([128, 512], bf16, tag="tm2")
                t3 = wkp.tile([128, 512], bf16, tag="tm3")
                for i, t in ((1, t1), (2, t2), (3, t3)):
                    nc.vector.tensor_tensor(
                        out=t[:, 0:gt], in0=xs(g, i * 2 + h, gt),
                        in1=pl(16 + i), op=OP.mult)
                nc.vector.tensor_add(t2[:, 0:gt], t2[:, 0:gt], t3[:, 0:gt])
                nc.vector.tensor_add(seg, seg, t1[:, 0:gt])
                nc.vector.tensor_add(seg, seg, t2[:, 0:gt])
            st[g]["xin"] = xin

        def stage_fo(g):
            """f_out^T = 2 * (w_inner @ x_in^T)  (POST_MULT folded)"""
            gt = SCS[g]
            xin = st[g]["xin"]
            fo = mxp.tile([128, 2 * gt], bf16, tag="fo", name=f"fo{g}")
            for ob in range(2):
                pf = psh.tile([128, VW], f32, tag="psh")
                for h in range(2):
                    nc.tensor.matmul(
                        out=pf[:, 0:gt],
                        lhsT=wit_sb[:, h * C + ob * 128:h * C + (ob + 1) * 128],
                        rhs=xin[:, h * gt:(h + 1) * gt],
                        start=(h == 0), stop=(h == 1))
                nc.scalar.mul(fo[:, ob * gt:(ob + 1) * gt], pf[:, 0:gt], POST_MULT)
            st[g]["fo"] = fo

        def stage_mix(g):
            """x_merge^T[kc] = sum_i res[o,i]*x^T[i,h] + post[o]*f_out^T[h]
            tree adds on DVE; gpsimd computes the independent f_out term.
            Two half-tiles so head chunks can start on kc 0..3."""
            gt = SCS[g]
            pl, fo = st[g]["pl"], st[g]["fo"]
            xmA = xmp.tile([128, 4 * gt], bf16, tag="xmA", name=f"xmA{g}")
            xmB = xmp.tile([128, 4 * gt], bf16, tag="xmB", name=f"xmB{g}")
            xmh = (xmA, xmB)
            for kc in range(NKC):
                o, h = kc // 2, kc % 2
                seg = xmh[kc // 4][:, (kc % 4) * gt:(kc % 4 + 1) * gt]
                nc.vector.tensor_tensor(
                    out=seg, in0=xs(g, h, gt), in1=pl(o * 4), op=OP.mult)
                t1 = wkp.tile([128, 512], bf16, tag="tm1")
                t2 = wkp.tile([128, 512], bf16, tag="tm2")
                t3 = wkp.tile([128, 512], bf16, tag="tm3")
                t4 = t4p.tile([128, 512], bf16, tag="tm4")
                for i, t in ((1, t1), (2, t2), (3, t3)):
                    nc.vector.tensor_tensor(
                        out=t[:, 0:gt], in0=xs(g, i * 2 + h, gt),
                        in1=pl(o * 4 + i), op=OP.mult)
                nc.gpsimd.tensor_tensor(
                    out=t4[:, 0:gt], in0=fo[:, h * gt:(h + 1) * gt],
                    in1=pl(20 + o), op=OP.mult)
                nc.vector.tensor_add(t2[:, 0:gt], t2[:, 0:gt], t3[:, 0:gt])
                nc.vector.tensor_add(seg, seg, t1[:, 0:gt])
                nc.vector.tensor_add(seg, seg, t2[:, 0:gt])
                nc.vector.tensor_add(seg, seg, t4[:, 0:gt])
            st[g]["xmg"] = xmh

        def head_chunk(g, tcx):
            gt = SCS[g]
            xmg = st[g]["xmg"]
            t0 = OFF[g] + tcx * 128
            stg = None
            for v in range(NV):
                w = min(VW, VS - v * VW)
                s, cv = v // 2, v % 2
                ph = psh.tile([128, VW], f32, tag="psh")
                for kc in range(NKC):
                    nc.tensor.matmul(
                        out=ph[:, 0:w],
                        lhsT=xmg[kc // 4][:, (kc % 4) * gt + tcx * 128:
                                          (kc % 4) * gt + (tcx + 1) * 128],
                        rhs=wt_t[(s, kc // 4)][:, kc % 4,
                                               cv * VW:cv * VW + w],
                        start=(kc == 0), stop=(kc == NKC - 1))
                half = v % 2
                if half == 0:
                    stg = stp.tile([128, 2 * VW], bf16, tag="stg")
                nc.scalar.copy(stg[:, half * VW:half * VW + w], ph[:, 0:w])
                if half == 1 or v == NV - 1:
                    v0 = v - half
                    ww = min(2 * VW, VS - v0 * VW)
                    nc.sync.dma_start(
                        out=out_p[t0:t0 + 128, v0 * VW:v0 * VW + ww],
                        in_=stg[:, 0:ww])

        # ---------------- emission (software pipeline) ----------------
        stage_lg(0)
        stage_lg(1)
        stage_x2(2)
        stage_planes(0)
        load_wt(3, nc.scalar)   # issues behind g0's bounces, lands ~40us
        stage_fo(0)
        stage_mix(0)
        stage_planes(1)
        load_wt(4, nc.scalar)   # lands ~45us, needed ~50us
        stage_fo(1)
        stage_mix(1)
        stage_lg(2)
        for g in range(NG):
            nch = SCS[g] // 128
            for tcx in range(nch):
                head_chunk(g, tcx)
                # g=0: lg(2)'s sinkhorn lands late in the DVE queue, so
                # its planes transposes would head-of-line block the PE
                # FIFO if emitted after chunk 0
                if tcx == (1 if g == 0 else 0) and g + 2 < NG:
                    stage_planes(g + 2)
                if tcx == min(1, nch - 1) and g + 2 < NG:
                    stage_fo(g + 2)
                    stage_mix(g + 2)
                if tcx == min(2, nch - 1) and g + 3 < NG:
                    stage_lg(g + 3)
            del st[g]
            xt_tiles.pop(g, None)

    if not nc.is_finalized():
        nc.finalize()
    return nc


_NC_CACHE = {}


def _get_nc():
    if "nc" not in _NC_CACHE:
        _NC_CACHE["nc"] = _build()
    return _NC_CACHE["nc"]


def _make_in_maps(input_ids, embed, w_inner, w_head, phi, b,
                  alpha_pre, alpha_post, alpha_res):
    import ml_dtypes
    bf = ml_dtypes.bfloat16

    ids = np.asarray(input_ids).reshape(-1).astype(np.int64)
    x = np.asarray(embed)[ids].astype(bf)                 # [NT, K]
    xt = np.ascontiguousarray(x.T)                        # [K, NT]
    # pack k-major chunks group-contiguously: xt3[p, 8*OFF[g]+kc*gt+t]
    xt3 = np.empty((128, NKC * NT), bf)
    for g in range(NG):
        gt, t0 = SCS[g], OFF[g]
        blk = xt[:, t0:t0 + gt].reshape(NKC, 128, gt).transpose(1, 0, 2)
        xt3[:, NKC * t0:NKC * (t0 + gt)] = blk.reshape(128, NKC * gt)

    phi_np = np.asarray(phi).astype(bf)                   # [K, M]
    phi3 = np.ascontiguousarray(
        phi_np.reshape(NKC, 128, M).transpose(1, 0, 2).reshape(128, NKC * M))
    witT = np.asarray(w_inner).astype(bf).T               # [ci, co]
    wit3 = np.ascontiguousarray(
        witT.reshape(2, 128, C).transpose(1, 0, 2).reshape(128, 2 * C))
    b_np = np.ascontiguousarray(np.asarray(b, dtype=np.float32).reshape(1, M))
    al = np.array([[np.asarray(alpha_pre).reshape(-1)[0],
                    np.asarray(alpha_post).reshape(-1)[0],
                    np.asarray(alpha_res).reshape(-1)[0]]], dtype=np.float32)
    wh = np.asarray(w_head).astype(bf)                    # [VOCAB, K]

    in_maps = []
    for i in range(NCORES):
        sl = wh[i * VS:(i + 1) * VS]                      # [<=VS, K]
        wvt = np.zeros((K, VS), bf)
        wvt[:, :sl.shape[0]] = sl.T
        # pack per (stripe, kc-half): wv4[p, OFFW[(s,h)] + kcq*cw + c]
        wv4 = np.empty((128, NKC * VS), bf)
        for s in range(NSTR):
            c0, cw = s * SW, SCW[s]
            for h in range(2):
                blk = wvt[h * 512:(h + 1) * 512, c0:c0 + cw]
                blk = blk.reshape(4, 128, cw).transpose(1, 0, 2)
                wv4[:, OFFW[(s, h)]:OFFW[(s, h)] + 4 * cw] = \
                    blk.reshape(128, 4 * cw)
        in_maps.append(dict(xt=xt3, wvt=np.ascontiguousarray(wv4),
                            wit=wit3, phi=phi3, b=b_np, al=al))
    return in_maps


def _run(in_maps, trace=False):
    from concourse.bass_utils import run_bass_kernel_spmd
    nc = _get_nc()
    return run_bass_kernel_spmd(nc, in_maps, list(range(NCORES)), trace=trace)


def kernel(input_ids, embed, w_inner, w_head, phi, b,
           alpha_pre, alpha_post, alpha_res):
    in_maps = _make_in_maps(input_ids, embed, w_inner, w_head, phi, b,
                            alpha_pre, alpha_post, alpha_res)
    res = _run(in_maps).results
    out = np.concatenate([np.asarray(res[i]["out"]) for i in range(NCORES)],
                         axis=1)[:, :VOCAB]
    return np.ascontiguousarray(out.reshape(B, S, VOCAB).astype(np.float32))


# revision 47
# speedup vs baseline: 1.0330x; 1.0330x over previous
"""Trainium2 Bass kernel for nn_MiniMHCLM (moe_routing).

Strategy (8 NeuronCores, SPMD, no collectives):
  - vocab-sharded head matmul: core i holds w_head rows [i*VS:(i+1)*VS]
    (host-sliced, zero-padded to uniform VS) transposed to k-major bf16;
    it computes logits for all 4096 tokens x its vocab slice and the host
    concatenates along vocab.
  - token embeddings are pre-gathered AND pre-transposed on the host into
    a k-major, group-packed layout so every device load is one contiguous
    DMA descriptor per tile.
  - EVERY bulk SBUF destination is split so each tile is written by
    exactly ONE DMA descriptor: w_head lives in 14 tiles (2 kc-halves x
    7 vocab stripes), x in 2 half-tiles per token group, the coefficient
    planes in 3 tiles. Tile-granular WAW tracking otherwise serializes
    the descriptors AND blocks the issuing engine's instruction stream.
  - per-token coeffs: phi-stationary matmul gives logits^T [24, T]; a
    col-tiled ones-matmul (PE column group 1, concurrent with the phi
    stream) produces sum(x^2) at PSUM partition 32; one PE transpose per
    128-token chunk moves logits+sumsq to token-major for the RMS scale,
    sigmoid/exp and 5 Sinkhorn iterations (converged vs the reference's
    8 to ~1e-3 absolute, far below the bf16 noise floor).
  - mixing runs transposed: coeffs are PE-transposed back to [24, T],
    bounced through DRAM in three pieces (pre/post first) and broadcast
    across partitions with stride-0 DMA reads; x_merge^T is built with
    tree-structured DVE multiply-adds (gpsimd computes the independent
    f_out*post term) and feeds the head matmul as stationary operand.
  - head matmul in bf16 with fp32 PSUM; PSUM evacuated by ACT copies to
    bf16 and DMA'd to DRAM bf16 (host converts to fp32).
  - flat software pipeline over 512-token groups with a 2-group prologue
    so the PE stream runs dense shortly after start with no ramp
    starvation and no HAM re-throttling.
  - DMA queue split: sync queue carries x/consts/w_head-stripes-0..3 and
    all output writes; scalar queue carries w_head stripes 4..6 and the
    coefficient bounces.
"""

import numpy as np

HC, C, TMAX = 4, 256, 8
TMAX_K = 5
RMS_EPS, PRE_EPS, SINK_EPS, POST_MULT = 1e-6, 1e-4, 1e-6, 2.0
VOCAB = 50257
B, S = 2, 2048
K = HC * C            # 1024
M = HC * HC + 2 * HC  # 24
NKC = K // 128        # 8 k-chunks
NCORES = 8
NT = B * S            # 4096
VS = 6283             # vocab rows per core (8*6283 = 50264 >= 50257)
VW = 512
NV = (VS + VW - 1) // VW          # 13 head tiles (12x512 + 139)
SW = 2 * VW                       # w stripe width (vocab cols)
NSTR = (VS + SW - 1) // SW        # 7 stripes
SCW = [min(SW, VS - s * SW) for s in range(NSTR)]
# wv4 col offset per (stripe, kc-half): each block is [128, 4*cw]
OFFW = {}
_off = 0
for _s in range(NSTR):
    for _h in range(2):
        OFFW[(_s, _h)] = _off
        _off += 4 * SCW[_s]
SCS = [256] + [512] * 7 + [256]   # token groups, sum = NT
OFF = [sum(SCS[:i]) for i in range(len(SCS))]
NG = len(SCS)
assert sum(SCS) == NT


def _build():
    from contextlib import ExitStack
    from concourse import bass, bacc, mybir
    import concourse.tile as tile
    from concourse.masks import make_identity

    f32 = mybir.dt.float32
    bf16 = mybir.dt.bfloat16
    f8e4 = mybir.dt.float8e4
    AX = mybir.AxisListType
    OP = mybir.AluOpType
    AF = mybir.ActivationFunctionType

    nc = bacc.Bacc(target_bir_lowering=False)
    xt_p = nc.declare_dram_parameter("xt", [128, NKC * NT], bf16, False)
    wvt_p = nc.declare_dram_parameter("wvt", [128, NKC * VS], bf16, False)
    wit_p = nc.declare_dram_parameter("wit", [128, 2 * C], bf16, False)
    phi_p = nc.declare_dram_parameter("phi", [128, NKC * M], bf16, False)
    b_p = nc.declare_dram_parameter("b", [1, M], f32, False)
    al_p = nc.declare_dram_parameter("al", [1, 3], f32, False)
    out_p = nc.declare_dram_parameter("out", [NT, VS], bf16, True)

    with ExitStack() as ctx:
        tc = ctx.enter_context(tile.TileContext(nc))
        const = ctx.enter_context(tc.tile_pool(name="const", bufs=1))
        wtp = ctx.enter_context(tc.tile_pool(name="wtp", bufs=1))
        xtp = ctx.enter_context(tc.tile_pool(name="xtp", bufs=3))
        lgp = ctx.enter_context(tc.tile_pool(name="lgp", bufs=2))
        cfp = ctx.enter_context(tc.tile_pool(name="cfp", bufs=2))
        plp = ctx.enter_context(tc.tile_pool(name="plp", bufs=1))
        mxp = ctx.enter_context(tc.tile_pool(name="mxp", bufs=2))
        xmp = ctx.enter_context(tc.tile_pool(name="xmp", bufs=3))
        wkp = ctx.enter_context(tc.tile_pool(name="wkp", bufs=1))
        t4p = ctx.enter_context(tc.tile_pool(name="t4p", bufs=2))
        x2p = ctx.enter_context(tc.tile_pool(name="x2p", bufs=1))
        stp = ctx.enter_context(tc.tile_pool(name="stp", bufs=2))
        psh = ctx.enter_context(tc.tile_pool(name="psh", bufs=4, space="PSUM"))
        psa = ctx.enter_context(tc.tile_pool(name="psa", bufs=2, space="PSUM"))
        pst = ctx.enter_context(tc.tile_pool(name="pst", bufs=1, space="PSUM"))
        psf = ctx.enter_context(tc.tile_pool(name="psf", bufs=1, space="PSUM"))
        drp = ctx.enter_context(tc.tile_pool(name="drp", bufs=2, space="DRAM"))

        # ---------------- constants (tiny, first) ----------------------
        identf = const.tile([128, 128], f32)
        make_identity(nc, identf[:])

        cst = const.tile([128, 2], f32)
        nc.vector.memset(cst[:, 0:1], 0.0)
        nc.vector.memset(cst[:, 1:2], RMS_EPS)
        zero_b = cst[:, 0:1]
        eps_b = cst[:, 1:2]

        ones = const.tile([128, 1], bf16)
        nc.vector.memset(ones[:], 1.0)

        phi_sb = const.tile([128, NKC * M], bf16)
        nc.sync.dma_start(out=phi_sb[:], in_=phi_p[:, :])
        b_bc = const.tile([128, M], f32)
        nc.sync.dma_start(out=b_bc[:], in_=b_p[0:1, :].to_broadcast([128, M]))
        al_bc = const.tile([128, 3], f32)
        nc.sync.dma_start(out=al_bc[:], in_=al_p[0:1, :].to_broadcast([128, 3]))
        wit_sb = const.tile([128, 2 * C], bf16)
        nc.sync.dma_start(out=wit_sb[:], in_=wit_p[:, :])

        # ---------------- input prefetch (2 tiles, 1 descriptor each) --
        xt_tiles = {}

        def prefetch_xt(g, eng=None):
            gt, t0 = SCS[g], OFF[g]
            halves = []
            for hh in range(2):
                xth = xtp.tile([128, 4 * gt], bf16, tag=f"xt{hh}",
                               name=f"xt{hh}_{g}")
                (eng or nc.sync).dma_start(
                    out=xth[:],
                    in_=xt_p[:, NKC * t0 + hh * 4 * gt:
                             NKC * t0 + (hh + 1) * 4 * gt])
                halves.append(xth)
            xt_tiles[g] = halves

        prefetch_xt(0)
        prefetch_xt(1)
        prefetch_xt(2)   # hoisted x2(2) consumes this early

        # ---------------- w_head^T: 14 tiles, 1 descriptor each --------
        # tile (s, h): [128, 4, cw] covering kc = 4h..4h+3 of stripe s
        wt_t = {}
        wt_sb = {}
        for s in range(NSTR):
            cw = SCW[s]
            for h in range(2):
                t = wtp.tile([128, 4 * cw], bf16, tag=f"wt_{s}_{h}")
                wt_sb[(s, h)] = t
                wt_t[(s, h)] = t[:].rearrange("p (k c) -> p k c", k=4)

        def load_wt(s, eng):
            for h in range(2):
                eng.dma_start(
                    out=wt_sb[(s, h)][:],
                    in_=wvt_p[:, OFFW[(s, h)]:OFFW[(s, h)] + 4 * SCW[s]])

        # 3-way queue split: s0-s3 sync, s4 scalar, s5-s6 on the gpsimd
        # swdge queue (an otherwise idle third channel)
        for s in (0, 1, 2, 3):
            load_wt(s, nc.sync)
        load_wt(4, nc.scalar)
        for s in (5, 6):
            load_wt(s, nc.gpsimd)
        prefetch_xt(3)

        st = {}  # per-group live tiles

        def xs(g, kc, gt):
            """xtg slice for k-chunk kc: [128, gt] view."""
            return xt_tiles_sb[g][kc // 4][:, (kc % 4) * gt:(kc % 4 + 1) * gt]

        xt_tiles_sb = xt_tiles  # alias (halves list per group)

        # ---------------- pipeline stages ----------------
        x2_tiles = {}

        def stage_x2(g):
            """squares for the sumsq matmuls (DVE), hoistable so the ss
            MMs never head-of-line block the PE FIFO on a late square."""
            gt = SCS[g]
            xh = xt_tiles[g]
            x2s = []
            for q4 in range(4):
                # fp8e4 with a x256 pre-scale: keeps the squares in fp8's
                # normal range; mean over K=1024 needs only ~0.1% accuracy
                x2 = x2p.tile([128, 2 * gt], f8e4, tag=f"x2q{q4}",
                              name=f"x2_{g}_{q4}")
                src = xh[q4 // 2][:, (q4 % 2) * 2 * gt:(q4 % 2 + 1) * 2 * gt]
                nc.vector.scalar_tensor_tensor(
                    out=x2[:], in0=src, scalar=256.0, in1=src,
                    op0=OP.mult, op1=OP.mult)
                x2s.append(x2)
            x2_tiles[g] = x2s

        def stage_lg(g):
            gt, t0 = SCS[g], OFF[g]
            nch = gt // 128
            if g >= 2 and g + 2 < NG:
                # gpsimd issue: a buffer-rotation wait here must not
                # block the sync engine's out-DMA stream
                prefetch_xt(g + 2, eng=nc.gpsimd)

            pslg = psa.tile([64, gt], f32, tag="pslg")
            if g not in x2_tiles:
                stage_x2(g)
            x2s = x2_tiles.pop(g)
            for kc in range(NKC):
                nc.tensor.matmul(
                    out=pslg[0:M, :],
                    lhsT=phi_sb[:, kc * M:(kc + 1) * M],
                    rhs=xs(g, kc, gt),
                    start=(kc == 0), stop=(kc == NKC - 1),
                    skip_group_check=True)
                nc.tensor.matmul(
                    out=pslg[32:33, :],
                    lhsT=ones[:],
                    rhs=x2s[kc // 2][:, (kc % 2) * gt:(kc % 2 + 1) * gt],
                    start=(kc == 0), stop=(kc == NKC - 1),
                    skip_group_check=True)

            lgsb = lgp.tile([33, gt], f32, tag="lgsb", name=f"lgsb{g}")
            nc.vector.memset(lgsb[0:33, :], 0.0)
            nc.scalar.copy(lgsb[0:M, :], pslg[0:M, :])
            nc.scalar.copy(lgsb[32:33, :], pslg[32:33, :])

            # token-major [128, nch, 24] + per-token sumsq column, via a
            # single 33-row PE transpose per 128-token chunk
            lgtm = lgp.tile([128, nch * 32], f32, tag="lgtm", name=f"lgtm{g}")
            msq = lgp.tile([128, nch], f32, tag="msq", name=f"msq{g}")
            for tcx in range(nch):
                pT = pst.tile([128, 128], f32, tag="psT")
                nc.tensor.transpose(
                    out=pT[:, 0:33],
                    in_=lgsb[0:33, tcx * 128:(tcx + 1) * 128],
                    identity=identf[0:33, 0:33])
                nc.scalar.copy(lgtm[:, tcx * 32:tcx * 32 + M], pT[:, 0:M])
                nc.scalar.copy(msq[:, tcx:tcx + 1], pT[:, 32:33])
            lgv = lgtm[:].rearrange("p (c w) -> p c w", w=32)

            scl = lgp.tile([128, nch], f32, tag="scl", name=f"scl{g}")
            nc.scalar.activation(out=scl[:], in_=msq[:],
                                 func=AF.Sqrt, scale=1.0 / (256.0 * K), bias=eps_b)
            nc.vector.reciprocal(scl[:], scl[:])
            for tcx in range(nch):
                nc.vector.tensor_scalar_mul(
                    lgv[:, tcx, 0:M], lgv[:, tcx, 0:M], scl[:, tcx:tcx + 1])
            nc.vector.tensor_tensor(
                out=lgv[:, :, 0:M], in0=lgv[:, :, 0:M],
                in1=b_bc[:][:, None, :].to_broadcast([128, nch, M]), op=OP.add)

            coefs = cfp.tile([128, nch * M], f32, tag="coefs",
                             name=f"coefs{g}")
            cfv = coefs[:].rearrange("p (c m) -> p c m", m=M)
            nc.scalar.activation(out=cfv[:, :, 16:20], in_=lgv[:, :, 0:4],
                                 func=AF.Sigmoid, bias=zero_b,
                                 scale=al_bc[:, 0:1])
            nc.vector.tensor_scalar_add(cfv[:, :, 16:20], cfv[:, :, 16:20],
                                        PRE_EPS)
            nc.scalar.activation(out=cfv[:, :, 20:24], in_=lgv[:, :, 4:8],
                                 func=AF.Sigmoid, bias=zero_b,
                                 scale=al_bc[:, 1:2])
            nc.scalar.activation(out=cfv[:, :, 0:16], in_=lgv[:, :, 8:24],
                                 func=AF.Exp, bias=zero_b, scale=al_bc[:, 2:3])

            mv4 = cfv[:, :, 0:16].rearrange("p c (o i) -> p c o i", i=4)
            mv4t = cfv[:, :, 0:16].rearrange("p c (o i) -> p c i o", i=4)
            for _ in range(TMAX_K):
                rs = wkp.tile([128, 16], f32, tag="rs")
                rsv = rs[:, 0:nch * 4].rearrange("p (c o) -> p c o", c=nch)
                nc.vector.tensor_reduce(rsv, mv4, axis=AX.X, op=OP.add)
                nc.vector.reciprocal(rs[:, 0:nch * 4], rs[:, 0:nch * 4])
                nc.vector.tensor_tensor(
                    out=mv4, in0=mv4,
                    in1=rsv[:, :, :, None].to_broadcast([128, nch, 4, 4]),
                    op=OP.mult)
                cs = wkp.tile([128, 16], f32, tag="cs")
                csv = cs[:, 0:nch * 4].rearrange("p (c i) -> p c i", c=nch)
                nc.vector.tensor_reduce(csv, mv4t, axis=AX.X, op=OP.add)
                nc.vector.reciprocal(cs[:, 0:nch * 4], cs[:, 0:nch * 4])
                nc.vector.tensor_tensor(
                    out=mv4, in0=mv4,
                    in1=csv[:, :, None, :].to_broadcast([128, nch, 4, 4]),
                    op=OP.mult)
            st[g] = dict(coefs=coefs)

        def stage_planes(g):
            """coefs -> [24, T] via PE transpose; bounce through DRAM in
            three pieces (pre/post first) and broadcast-read into three
            single-descriptor plane tiles; build x_in^T from pre planes."""
            gt = SCS[g]
            nch = gt // 128
            coefs = st[g]["coefs"]
            ctstg = cfp.tile([32, gt], bf16, tag="ctstg", name=f"ctstg{g}")
            for tcx in range(nch):
                pT = pst.tile([128, 128], f32, tag="psT")
                nc.tensor.transpose(
                    out=pT[0:M, 0:128],
                    in_=coefs[:, tcx * M:(tcx + 1) * M],
                    identity=identf[:, 0:128])
                nc.scalar.copy(
                    ctstg[0:M, tcx * 128:(tcx + 1) * 128], pT[0:M, 0:128])
            planA = plp.tile([128, 8 * gt], bf16, tag="planA",
                             name=f"planA{g}")   # planes 16..23 (pre+post)
            planB1 = plp.tile([128, 8 * gt], bf16, tag="planB1",
                              name=f"planB1{g}")  # res planes 0..7
            planB2 = plp.tile([128, 8 * gt], bf16, tag="planB2",
                              name=f"planB2{g}")  # res planes 8..15
            for nmA, rows, dst in (("A", (16, 24), planA),
                                   ("B1", (0, 8), planB1),
                                   ("B2", (8, 16), planB2)):
                dt = drp.tile([1, 8 * gt], bf16, tag=f"dt{nmA}",
                              name=f"dt{nmA}{g}")
                nc.scalar.dma_start(
                    out=dt[0:1, :].rearrange("x (c t) -> (x c) t", c=8),
                    in_=ctstg[rows[0]:rows[1], :])
                nc.scalar.dma_start(
                    out=dst[:], in_=dt[0:1, :].to_broadcast([128, 8 * gt]))

            def pl(idx):
                if idx >= 16:
                    return planA[:, (idx - 16) * gt:(idx - 15) * gt]
                if idx < 8:
                    return planB1[:, idx * gt:(idx + 1) * gt]
                return planB2[:, (idx - 8) * gt:(idx - 7) * gt]

            st[g]["pl"] = pl
            # x_in^T = sum_i h_pre[i] * x^T[i]
            xin = mxp.tile([128, 2 * gt], bf16, tag="xin", name=f"xin{g}")
            for h in range(2):
                seg = xin[:, h * gt:(h + 1) * gt]
                nc.vector.tensor_tensor(
                    out=seg, in0=xs(g, h, gt), in1=pl(16), op=OP.mult)
                t1 = wkp.tile([128, 512], bf16, tag="tm1")
                t2 = wkp.tile([128, 512], bf16, tag="tm2")
                t3 = wkp.tile([128, 512], bf16, tag="tm3")
                for i, t in ((1, t1), (2, t2), (3, t3)):
                    nc.vector.tensor_tensor(
                        out=t[:, 0:gt], in0=xs(g, i * 2 + h, gt),
                        in1=pl(16 + i), op=OP.mult)
                nc.vector.tensor_add(t2[:, 0:gt], t2[:, 0:gt], t3[:, 0:gt])
                nc.vector.tensor_add(seg, seg, t1[:, 0:gt])
                nc.vector.tensor_add(seg, seg, t2[:, 0:gt])
            st[g]["xin"] = xin

        def stage_fo(g):
            """f_out^T = 2 * (w_inner @ x_in^T)  (POST_MULT folded)"""
            gt = SCS[g]
            xin = st[g]["xin"]
            fo = mxp.tile([128, 2 * gt], bf16, tag="fo", name=f"fo{g}")
            for ob in range(2):
                pf = psf.tile([128, VW], f32, tag="psf")
                for h in range(2):
                    nc.tensor.matmul(
                        out=pf[:, 0:gt],
                        lhsT=wit_sb[:, h * C + ob * 128:h * C + (ob + 1) * 128],
                        rhs=xin[:, h * gt:(h + 1) * gt],
                        start=(h == 0), stop=(h == 1))
                nc.scalar.mul(fo[:, ob * gt:(ob + 1) * gt], pf[:, 0:gt], POST_MULT)
            st[g]["fo"] = fo

        def stage_mix(g):
            """x_merge^T[kc] = sum_i res[o,i]*x^T[i,h] + post[o]*f_out^T[h]
            tree adds on DVE; gpsimd computes the independent f_out term.
            Two half-tiles so head chunks can start on kc 0..3."""
            gt = SCS[g]
            pl, fo = st[g]["pl"], st[g]["fo"]
            xmA = xmp.tile([128, 4 * gt], bf16, tag="xmA", name=f"xmA{g}")
            xmB = xmp.tile([128, 4 * gt], bf16, tag="xmB", name=f"xmB{g}")
            xmh = (xmA, xmB)
            for kc in range(NKC):
                o, h = kc // 2, kc % 2
                seg = xmh[kc // 4][:, (kc % 4) * gt:(kc % 4 + 1) * gt]
                nc.vector.tensor_tensor(
                    out=seg, in0=xs(g, h, gt), in1=pl(o * 4), op=OP.mult)
                t1 = wkp.tile([128, 512], bf16, tag="tm1")
                t2 = wkp.tile([128, 512], bf16, tag="tm2")
                t3 = wkp.tile([128, 512], bf16, tag="tm3")
                t4 = t4p.tile([128, 512], bf16, tag="tm4")
                for i, t in ((1, t1), (2, t2), (3, t3)):
                    nc.vector.tensor_tensor(
                        out=t[:, 0:gt], in0=xs(g, i * 2 + h, gt),
                        in1=pl(o * 4 + i), op=OP.mult)
                nc.gpsimd.tensor_tensor(
                    out=t4[:, 0:gt], in0=fo[:, h * gt:(h + 1) * gt],
                    in1=pl(20 + o), op=OP.mult)
                nc.vector.tensor_add(t2[:, 0:gt], t2[:, 0:gt], t3[:, 0:gt])
                nc.vector.tensor_add(seg, seg, t1[:, 0:gt])
                nc.vector.tensor_add(seg, seg, t2[:, 0:gt])
                nc.vector.tensor_add(seg, seg, t4[:, 0:gt])
            st[g]["xmg"] = xmh

        def head_chunk(g, tcx):
            gt = SCS[g]
            xmg = st[g]["xmg"]
            t0 = OFF[g] + tcx * 128
            stg = None
            for v in range(NV):
                w = min(VW, VS - v * VW)
                s, cv = v // 2, v % 2
                ph = psh.tile([128, VW], f32, tag="psh")
                for kc in range(NKC):
                    nc.tensor.matmul(
                        out=ph[:, 0:w],
                        lhsT=xmg[kc // 4][:, (kc % 4) * gt + tcx * 128:
                                          (kc % 4) * gt + (tcx + 1) * 128],
                        rhs=wt_t[(s, kc // 4)][:, kc % 4,
                                               cv * VW:cv * VW + w],
                        start=(kc == 0), stop=(kc == NKC - 1))
                half = v % 2
                if half == 0:
                    stg = stp.tile([128, 2 * VW], bf16, tag="stg")
                nc.scalar.copy(stg[:, half * VW:half * VW + w], ph[:, 0:w])
                if half == 1 or v == NV - 1:
                    v0 = v - half
                    ww = min(2 * VW, VS - v0 * VW)
                    nc.sync.dma_start(
                        out=out_p[t0:t0 + 128, v0 * VW:v0 * VW + ww],
                        in_=stg[:, 0:ww])

        # ---------------- emission (software pipeline) ----------------
        stage_lg(0)
        stage_lg(1)
        stage_planes(0)
        stage_fo(0)
        stage_mix(0)
        stage_lg(2)
        stage_planes(1)
        stage_fo(1)
        stage_mix(1)
        for g in range(NG):
            nch = SCS[g] // 128
            for tcx in range(nch):
                head_chunk(g, tcx)
                if tcx == 0 and g + 2 < NG:
                    stage_planes(g + 2)
                if tcx == min(1, nch - 1) and g + 2 < NG:
                    stage_fo(g + 2)
                    stage_mix(g + 2)
                if tcx == min(2, nch - 1) and g + 3 < NG:
                    stage_lg(g + 3)
            del st[g]
            xt_tiles.pop(g, None)

    if not nc.is_finalized():
        nc.finalize()
    return nc


_NC_CACHE = {}


def _get_nc():
    if "nc" not in _NC_CACHE:
        _NC_CACHE["nc"] = _build()
    return _NC_CACHE["nc"]


def _make_in_maps(input_ids, embed, w_inner, w_head, phi, b,
                  alpha_pre, alpha_post, alpha_res):
    import ml_dtypes
    bf = ml_dtypes.bfloat16

    ids = np.asarray(input_ids).reshape(-1).astype(np.int64)
    x = np.asarray(embed)[ids].astype(bf)                 # [NT, K]
    xt = np.ascontiguousarray(x.T)                        # [K, NT]
    # pack k-major chunks group-contiguously: xt3[p, 8*OFF[g]+kc*gt+t]
    xt3 = np.empty((128, NKC * NT), bf)
    for g in range(NG):
        gt, t0 = SCS[g], OFF[g]
        blk = xt[:, t0:t0 + gt].reshape(NKC, 128, gt).transpose(1, 0, 2)
        xt3[:, NKC * t0:NKC * (t0 + gt)] = blk.reshape(128, NKC * gt)

    phi_np = np.asarray(phi).astype(bf)                   # [K, M]
    phi3 = np.ascontiguousarray(
        phi_np.reshape(NKC, 128, M).transpose(1, 0, 2).reshape(128, NKC * M))
    witT = np.asarray(w_inner).astype(bf).T               # [ci, co]
    wit3 = np.ascontiguousarray(
        witT.reshape(2, 128, C).transpose(1, 0, 2).reshape(128, 2 * C))
    b_np = np.ascontiguousarray(np.asarray(b, dtype=np.float32).reshape(1, M))
    al = np.array([[np.asarray(alpha_pre).reshape(-1)[0],
                    np.asarray(alpha_post).reshape(-1)[0],
                    np.asarray(alpha_res).reshape(-1)[0]]], dtype=np.float32)
    wh = np.asarray(w_head).astype(bf)                    # [VOCAB, K]

    in_maps = []
    for i in range(NCORES):
        sl = wh[i * VS:(i + 1) * VS]                      # [<=VS, K]
        wvt = np.zeros((K, VS), bf)
        wvt[:, :sl.shape[0]] = sl.T
        # pack per (stripe, kc-half): wv4[p, OFFW[(s,h)] + kcq*cw + c]
        wv4 = np.empty((128, NKC * VS), bf)
        for s in range(NSTR):
            c0, cw = s * SW, SCW[s]
            for h in range(2):
                blk = wvt[h * 512:(h + 1) * 512, c0:c0 + cw]
                blk = blk.reshape(4, 128, cw).transpose(1, 0, 2)
                wv4[:, OFFW[(s, h)]:OFFW[(s, h)] + 4 * cw] = \
                    blk.reshape(128, 4 * cw)
        in_maps.append(dict(xt=xt3, wvt=np.ascontiguousarray(wv4),
                            wit=wit3, phi=phi3, b=b_np, al=al))
    return in_maps


def _run(in_maps, trace=False):
    from concourse.bass_utils import run_bass_kernel_spmd
    nc = _get_nc()
    return run_bass_kernel_spmd(nc, in_maps, list(range(NCORES)), trace=trace)


def kernel(input_ids, embed, w_inner, w_head, phi, b,
           alpha_pre, alpha_post, alpha_res):
    in_maps = _make_in_maps(input_ids, embed, w_inner, w_head, phi, b,
                            alpha_pre, alpha_post, alpha_res)
    res = _run(in_maps).results
    out = np.concatenate([np.asarray(res[i]["out"]) for i in range(NCORES)],
                         axis=1)[:, :VOCAB]
    return np.ascontiguousarray(out.reshape(B, S, VOCAB).astype(np.float32))
